# revision 40
# baseline (speedup 1.0000x reference)
"""Trainium2 Bass kernel for nn_Attention_15470472200471.

Sharding (8 cores): core c -> batch c//2, head-half c%2 (8 of 16 heads).
Host: layernorm stats fold (exact fp32), concat memories, transpose, bf16 cast.
Device (per core): K/V projections first, then per 512-query chunk: Q
projection, dots^T = K @ Q^T (row-tile-paired heads into one 1024-wide PSUM
tile), exp via ScalarE (12/17 nk tiles) + Schraudolph fast-exp on VectorE
(5/17 tiles, int16 bitcast-to-bf16), AV interleaved 4 tiles behind dots with
a 64-wide ones block appended to V so the softmax denominator comes out of
the PE replicated across partitions (no broadcast), normalize, out-projection
spread across the next chunk's blocks, bf16 output.
Host: sum the two head-half partials per batch + bo in fp32.
"""

import numpy as np
import ml_dtypes

B, N, DIM = 4, 2048, 1024
HEADS, DHEAD = 16, 64
N_MEM = 64
NK = N + N_MEM            # 2112
HL = 8                    # local heads per core
IL = HL * DHEAD           # 512 local inner dim
P = 128
NCORES = 8
KC = DIM // P             # 8 contraction chunks over model dim
MCQ = IL // P             # 4 partition-chunks over local inner
SC = N // 512             # 4 seq chunks of 512
NKT = (NK + P - 1) // P   # 17 nk tiles (16 full + 1 of 64)

# nk tiles whose exp runs on VectorE via Schraudolph fast-exp instead of
# ScalarE exact exp. More tiles -> faster but less accurate (~0.3%/tile).
DVE_EXP_TILES = (3, 6, 9, 12, 15)
# bf16 Schraudolph: bitcast(int16(x*(128/ln2) + B)) ~= 2^(x/ln2) = e^x.
# Dots psum holds raw q.k, exp arg is score/8: fold 0.125 into the scale.
FEXP_SCALE = 16.0 / float(np.log(2.0))     # 23.0831...
FEXP_BIAS = 16249.0                        # calibrated for truncating convert

_CACHE = {}


def _build_nc():
    import concourse.mybir as mybir
    import concourse.tile as tile
    from concourse import bacc

    f32 = mybir.dt.float32
    bf16 = mybir.dt.bfloat16
    i16 = mybir.dt.int16
    EXPF = mybir.ActivationFunctionType.Exp
    MULT = mybir.AluOpType.mult
    ADD = mybir.AluOpType.add

    nc = bacc.Bacc("TRN2", target_bir_lowering=False, debug=False)
    xkvT_d = nc.dram_tensor("xkvT", [DIM, NK], bf16, kind="ExternalInput")
    wq_d = nc.dram_tensor("wq", [DIM, IL], bf16, kind="ExternalInput")
    wk_d = nc.dram_tensor("wk", [DIM, IL], bf16, kind="ExternalInput")
    wv_d = nc.dram_tensor("wv", [DIM, IL], bf16, kind="ExternalInput")
    wo_d = nc.dram_tensor("wo", [IL, DIM], bf16, kind="ExternalInput")
    out_d = nc.dram_tensor("out", [N, DIM], bf16, kind="ExternalOutput")

    # x / K-proj column chunks over NK: 3x512 + 576; V tiles 4c..(4c+3|4c+4)
    x_chunks = [(0, 512), (512, 512), (1024, 512), (1536, 576)]

    with tile.TileContext(nc) as tc:
        with (
            tc.tile_pool(name="big", bufs=1) as big,
            tc.tile_pool(name="xpt", bufs=2) as xpt,
            tc.tile_pool(name="otp", bufs=2) as otp,
            tc.tile_pool(name="rcp", bufs=2) as rcp,
            tc.tile_pool(name="outb", bufs=3) as outb,
            tc.tile_pool(name="ps", bufs=2, space="PSUM") as psp,
            tc.tile_pool(name="psO", bufs=2, space="PSUM") as psO,
            tc.tile_pool(name="psZ", bufs=2, space="PSUM") as psZ,
        ):
            # ---- persistent tensors, chunked loads in consumption order ----
            # x shares a pool with the pt tiles: x is dead after the
            # projections, so pt round-robins into its buffer.
            x_sb = xpt.tile([P, KC, NK], bf16, tag="pt")
            wk_sb = big.tile([P, KC, IL], bf16, tag="wk")
            wv_sb = big.tile([P, KC, IL], bf16, tag="wv")
            wq_sb = big.tile([P, KC, IL], bf16, tag="wq")
            wo_sb = big.tile([P, MCQ, DIM], bf16, tag="wo")

            def load_x(ci, k0, k1):
                o, w = x_chunks[ci]
                nc.sync.dma_start(
                    x_sb[:, k0:k1, o:o + w],
                    xkvT_d[k0 * P:k1 * P, o:o + w].rearrange(
                        "(c p) n -> p c n", p=P
                    ),
                )

            def load_w(sb, dram, lo, hi, k0=0, k1=KC):
                nc.scalar.dma_start(
                    sb[:, k0:k1, lo:hi],
                    dram[k0 * P:k1 * P, lo:hi].rearrange("(c p) n -> p c n", p=P),
                )

            # x chunks on the SP HWDGE queue; weights on the ACT HWDGE queue
            # (parallel queues, ACT is idle during the load phase). The first
            # chunk/m is split by contraction half so the first matmuls of the
            # K projection can start one half-DMA earlier.
            load_x(0, 0, 4)
            load_x(0, 4, 8)
            load_x(1, 0, KC)
            load_x(2, 0, KC)
            load_w(wk_sb, wk_d, 0, P, 0, 4)
            load_w(wk_sb, wk_d, 0, P, 4, 8)
            for m in range(1, MCQ):
                load_w(wk_sb, wk_d, m * P, (m + 1) * P)
            # the last x chunk rides the (shorter) ACT queue behind wk
            o3, w3 = x_chunks[3]
            nc.scalar.dma_start(
                x_sb[:, :, o3:o3 + w3],
                xkvT_d[:, o3:o3 + w3].rearrange("(c p) n -> p c n", p=P),
            )
            for h in range(2):
                nc.scalar.dma_start(
                    wv_sb[:, 4 * h:4 * h + 4, :],
                    wv_d[4 * h * P:(4 * h + 4) * P, :].rearrange(
                        "(c p) n -> p c n", p=P
                    ),
                )
            for m in range(MCQ):
                load_w(wq_sb, wq_d, m * P, (m + 1) * P)
            for d in range(2):
                nc.scalar.dma_start(
                    wo_sb[:, :, d * 512:(d + 1) * 512],
                    wo_d[:, d * 512:(d + 1) * 512].rearrange(
                        "(c p) n -> p c n", p=P
                    ),
                )

            qt_sb = big.tile([P, MCQ, N], bf16, tag="qt")        # Q^T
            kt_sb = big.tile([P, MCQ, NK], bf16, tag="kt")       # K^T
            # per head: [V_h (64 cols) | ones (64 cols)] so the AV matmul
            # replicates the softmax denominator across 64 psum partitions
            vaug_sb = big.tile([P, NKT, HL, 128], bf16, tag="va")
            nc.vector.memset(vaug_sb[:, :, :, 64:128], 1.0)

            # ---- K^T projection, per x column chunk ----
            # (psum matmul groups must stay within one 512-col bank)
            for ci, (o, w) in enumerate(x_chunks):
                for m in range(MCQ):
                    ps = psp.tile([P, 1024], f32, tag="d")
                    for (so, sw) in ((0, 512), (512, w - 512)) if w > 512 \
                            else ((0, w),):
                        for k in range(KC):
                            nc.tensor.matmul(
                                ps[:, so:so + sw],
                                wk_sb[:, k, m * P:(m + 1) * P],
                                x_sb[:, k, o + so:o + so + sw],
                                start=(k == 0), stop=(k == KC - 1),
                            )
                    nc.vector.tensor_copy(
                        out=kt_sb[:, m, o:o + w], in_=ps[:, :w]
                    )
            # ---- V projection, into vaug columns ----
            for ta in range(0, NKT, 2):
                tb = min(ta + 2, NKT)
                ps = psp.tile([P, 1024], f32, tag="d")
                for ti in range(ta, tb):
                    mt = P if ti < 16 else 64
                    c0 = (ti - ta) * 512
                    for k in range(KC):
                        nc.tensor.matmul(
                            ps[:mt, c0:c0 + 512],
                            x_sb[:, k, ti * P:ti * P + mt],
                            wv_sb[:, k, 0:IL],
                            start=(k == 0), stop=(k == KC - 1),
                        )
                for ti in range(ta, tb):
                    mt = P if ti < 16 else 64
                    c0 = (ti - ta) * 512
                    nc.vector.tensor_copy(
                        out=vaug_sb[:mt, ti, :, 0:64],
                        in_=ps[:mt, c0:c0 + 512].rearrange(
                            "p (h e) -> p h e", e=64
                        ),
                    )
            # ---- Q^T projection, all seq chunks ----
            for s in range(SC):
                for ma in range(0, MCQ, 2):
                    ps = psp.tile([P, 1024], f32, tag="d")
                    for mi in range(2):
                        c0 = mi * 512
                        for k in range(KC):
                            nc.tensor.matmul(
                                ps[:, c0:c0 + 512],
                                wq_sb[:, k, (ma + mi) * P:(ma + mi + 1) * P],
                                x_sb[:, k, s * 512:(s + 1) * 512],
                                start=(k == 0), stop=(k == KC - 1),
                            )
                    nc.vector.tensor_copy(
                        out=qt_sb[:, ma:ma + 2, s * 512:(s + 1) * 512],
                        in_=ps.rearrange("p (m n) -> p m n", n=512),
                    )

            # ---- per seq chunk: attention, with out-proj of the previous
            # chunk's rows emitted spread across this chunk's head-pair
            # blocks (fills the PE while normalize chains drain) ----
            LAG = 4

            def out_proj(s_, st):
                r0 = s_ * 512 + st * P
                pz = psZ.tile([P, 512], f32, tag="z")
                pz2 = psZ.tile([P, 512], f32, tag="z")
                for d, tile_ in ((0, pz), (1, pz2)):
                    for ic in range(MCQ):
                        nc.tensor.matmul(
                            tile_,
                            ot_tiles[s_][:, ic, st * P:(st + 1) * P],
                            wo_sb[:, ic, d * 512:(d + 1) * 512],
                            start=(ic == 0), stop=(ic == MCQ - 1),
                        )
                ob = outb.tile([P, 1024], bf16, tag="ob")
                nc.vector.tensor_copy(out=ob[:, 0:512], in_=pz)
                nc.vector.tensor_copy(out=ob[:, 512:1024], in_=pz2)
                nc.sync.dma_start(out_d[r0:r0 + P, :], ob)

            ot_tiles = {}
            for s in range(SC):
                ot_sb = otp.tile([P, MCQ, 512], bf16, tag="ot")
                ot_tiles[s] = ot_sb
                for pr in range(MCQ):  # head pair (2pr, 2pr+1)
                    pt = xpt.tile([P, NKT, 1024], bf16, tag="pt")
                    po_a = psO.tile([P, 512], f32, tag="o")
                    po_b = psO.tile([P, 512], f32, tag="o")

                    def av_step(t):
                        mt = P if t < 16 else 64
                        for hh, po in ((0, po_a), (1, po_b)):
                            h = 2 * pr + hh
                            nc.tensor.matmul(
                                po,
                                vaug_sb[:mt, t, h],
                                pt[:mt, t, hh * 512:hh * 512 + 512],
                                start=(t == 0), stop=(t == NKT - 1),
                            )

                    for t in range(NKT):
                        mt = P if t < 16 else 64
                        ps = psp.tile([P, 1024], f32, tag="d")
                        nc.tensor.matmul(
                            ps[:mt, 0:512],
                            kt_sb[0:64, pr, t * P:t * P + mt],
                            qt_sb[0:64, pr, s * 512:(s + 1) * 512],
                            start=True, stop=True,
                        )
                        nc.tensor.matmul(
                            ps[:mt, 512:1024],
                            kt_sb[64:128, pr, t * P:t * P + mt],
                            qt_sb[64:128, pr, s * 512:(s + 1) * 512],
                            start=True, stop=True,
                        )
                        if t in DVE_EXP_TILES:
                            nc.vector.tensor_scalar(
                                pt[:mt, t].bitcast(i16), ps[:mt],
                                FEXP_SCALE, FEXP_BIAS, MULT, ADD,
                            )
                        else:
                            nc.scalar.activation(
                                pt[:mt, t], ps[:mt], EXPF, scale=0.125
                            )
                        if t >= LAG:
                            av_step(t - LAG)
                        if t == 8 and s > 0:
                            out_proj(s - 1, pr)
                    for t in range(NKT - LAG, NKT):
                        av_step(t)
                    for hh, po in ((0, po_a), (1, po_b)):
                        rec = rcp.tile([64, 512], f32, tag="rec")
                        nc.vector.reciprocal(rec, po[64:128, 0:512])
                        nc.vector.tensor_mul(
                            out=ot_sb[hh * 64:hh * 64 + 64, pr, :],
                            in0=po[0:64],
                            in1=rec,
                        )
                    if s == SC - 1 and pr == 3:
                        # final chunk: pre-accumulate out-proj partials for
                        # st 0 over the head pairs already normalized, so
                        # only the ic=3 matmuls remain after the last pair
                        tpa = psZ.tile([P, 512], f32, tag="z")
                        tpb = psZ.tile([P, 512], f32, tag="z")
                        tail_pz = (tpa, tpb)
                        for d, tile_ in ((0, tail_pz[0]), (1, tail_pz[1])):
                            for ic in range(3):
                                nc.tensor.matmul(
                                    tile_,
                                    ot_sb[:, ic, 0:P],
                                    wo_sb[:, ic, d * 512:(d + 1) * 512],
                                    start=(ic == 0), stop=False,
                                )
            s_ = SC - 1
            for d, tile_ in ((0, tail_pz[0]), (1, tail_pz[1])):
                nc.tensor.matmul(
                    tile_,
                    ot_tiles[s_][:, 3, 0:P],
                    wo_sb[:, 3, d * 512:(d + 1) * 512],
                    start=False, stop=True,
                )
            ob = outb.tile([P, 1024], bf16, tag="ob")
            nc.vector.tensor_copy(out=ob[:, 0:512], in_=tail_pz[0])
            nc.vector.tensor_copy(out=ob[:, 512:1024], in_=tail_pz[1])
            nc.sync.dma_start(out_d[s_ * 512:s_ * 512 + P, :], ob)
            for st in range(1, 4):
                out_proj(SC - 1, st)
    nc.compile()
    return nc


def kernel(**inputs):
    x = np.asarray(inputs["x"], np.float32)
    memories = np.asarray(inputs["memories"], np.float32)
    g = np.asarray(inputs["ln_gamma"], np.float32)
    beta = np.asarray(inputs["ln_beta"], np.float32)
    Wq = np.asarray(inputs["Wq"], np.float32)
    Wkv = np.asarray(inputs["Wkv"], np.float32)
    Wo = np.asarray(inputs["Wo"], np.float32)
    bo = np.asarray(inputs["bo"], np.float32)

    mu = x.mean(-1, keepdims=True)
    var = x.var(-1, keepdims=True)
    xn = (x - mu) / np.sqrt(var + 1e-5) * g + beta

    bf = ml_dtypes.bfloat16
    in_maps = []
    for c in range(NCORES):
        bb, half = c // 2, c % 2
        i0 = half * IL
        xkv = np.concatenate([xn[bb], memories], axis=0)  # [NK, DIM]
        in_maps.append({
            "xkvT": np.ascontiguousarray(xkv.T).astype(bf),
            "wq": np.ascontiguousarray(Wq[:, i0:i0 + IL]).astype(bf),
            "wk": np.ascontiguousarray(Wkv[:, i0:i0 + IL]).astype(bf),
            "wv": np.ascontiguousarray(Wkv[:, DIM + i0:DIM + i0 + IL]).astype(bf),
            "wo": np.ascontiguousarray(Wo[i0:i0 + IL, :]).astype(bf),
        })

    if "nc" not in _CACHE:
        _CACHE["nc"] = _build_nc()
    nc = _CACHE["nc"]

    import time as _time
    from concourse.bass_utils import run_bass_kernel_spmd
    t0 = _time.time()
    res = run_bass_kernel_spmd(nc, in_maps, list(range(NCORES)))
    t1 = _time.time()
    if getattr(res, "exec_time_ns", None):
        print(f"HW exec time: {res.exec_time_ns} ns")
    else:
        print(f"spmd call wall: {(t1 - t0) * 1e9:.0f} ns")

    out = np.empty((B, N, DIM), np.float32)
    for bb in range(B):
        out[bb] = (
            np.asarray(res.results[2 * bb]["out"], np.float32)
            + np.asarray(res.results[2 * bb + 1]["out"], np.float32)
            + bo
        )
    return out
